# revision 1
# baseline (speedup 1.0000x reference)
"""Trainium2 Bass kernel for ContinuousConv1DSim (gnn_message_passing).

Reformulation (validated vs reference in fp32 numpy, rel err ~4e-5):
  G = F * npm (per-l mask), H = G * t
  MM1  (PE): psw[c2, l] = sum_j GH[j, c2] * Band[j, l]   -- causal 8-wide window
             sums over l, output TRANSPOSED (channels on partitions), with a
             second accumulating matmul adding the previous tile's halo rows.
  MM2a (PE): psp[l, 0:64]  = A_e   (window(G) @ W^T)
             psp[l, 64:128]= D_raw (window(H) @ W^T - window(G) @ bias)
  MM2b (PE): pssp[l, s*64+o] = u[s] * A_e[l, o]          -- s-expansion on PE
  sim_m   = (A_m * t - D_m) with A_m/D_m = npm * psp     (ACT copy w/ scale)
  obuf_sim= pssp * udt + sim_m (broadcast over s)        -- one DVE STT
  real[l] = npm[l] * (t[l] * A_m[l-1] - D_m[l-1])        -- partition-shifted STT
Output rows per l: [real, sim + u_s * udt * A] for s=0..7, last row real[L-1].

Pure data parallel: batch 32 -> 8 cores x 4. All params replicated.
"""

import numpy as np

B, L, C, O, S = 32, 2048, 64, 64, 8
NCORES = 8
BPC = B // NCORES          # 4 batches per core
NT = L // 128              # 16 l-tiles per batch
ROWS = (L - 1) * (S + 1) + 1  # 18424
F32 = None  # set after mybir import


def _consts(W, bias, u):
    n = np.arange(128)
    bandc = ((n[:, None] >= n[None, :] - 7) & (n[:, None] <= n[None, :])).astype(np.float32)
    bandp = (n[:, None] >= n[None, :] + 121).astype(np.float32)
    prba = np.zeros((128, 128), np.float32)
    prba[0:64, 0:64] = W.T           # A_e from U
    prba[0:64, 64:128] = -bias       # -F_e into D_raw
    prba[64:128, 64:128] = W.T       # TA_e into D_raw
    prbb = np.zeros((128, 512), np.float32)
    for s in range(S):
        prbb[0:64, s * 64:(s + 1) * 64] = u[s] * W.T
    return bandc, bandp, prba, prbb


def _build_nc():
    import concourse.bass as bass
    import concourse.bacc as bacc
    import concourse.mybir as mybir
    import concourse.tile as tile

    f32 = mybir.dt.float32
    Copy = mybir.ActivationFunctionType.Copy
    mult = mybir.AluOpType.mult
    sub = mybir.AluOpType.subtract
    add = mybir.AluOpType.add

    nc = bacc.Bacc("TRN2", target_bir_lowering=False, debug=False,
                   num_devices=NCORES)

    FD = nc.dram_tensor("f", [BPC, L, C], f32, kind="ExternalInput").ap()
    TSD = nc.dram_tensor("ts", [BPC, L + 128], f32, kind="ExternalInput").ap()
    UDD = nc.dram_tensor("ud", [BPC, L], f32, kind="ExternalInput").ap()
    NPD = nc.dram_tensor("np", [BPC, L + 128], f32, kind="ExternalInput").ap()
    BCD = nc.dram_tensor("bandc", [128, 128], f32, kind="ExternalInput").ap()
    BPD = nc.dram_tensor("bandp", [128, 128], f32, kind="ExternalInput").ap()
    PAD = nc.dram_tensor("prba", [128, 128], f32, kind="ExternalInput").ap()
    PBD = nc.dram_tensor("prbb", [128, 512], f32, kind="ExternalInput").ap()
    OUTD = nc.dram_tensor("out", [BPC, ROWS, O], f32, kind="ExternalOutput").ap()

    with tile.TileContext(nc) as tc:
        with (
            tc.tile_pool(name="const", bufs=1) as cpool,
            tc.tile_pool(name="scal", bufs=2) as spool,
            tc.tile_pool(name="feat", bufs=3) as fpool,
            tc.tile_pool(name="gh", bufs=3) as ghpool,
            tc.tile_pool(name="sbw", bufs=3) as sbwpool,
            tc.tile_pool(name="pp", bufs=3) as pppool,
            tc.tile_pool(name="simm", bufs=3) as simpool,
            tc.tile_pool(name="ob", bufs=3) as obpool,
            tc.tile_pool(name="ro", bufs=3) as ropool,
            tc.tile_pool(name="psw", bufs=3, space=bass.MemorySpace.PSUM) as pwpool,
            tc.tile_pool(name="psp", bufs=2, space=bass.MemorySpace.PSUM) as papool,
            tc.tile_pool(name="pssp", bufs=2, space=bass.MemorySpace.PSUM) as pbpool,
        ):
            bandc_t = cpool.tile([128, 128], f32, tag="bandc")
            bandp_t = cpool.tile([128, 128], f32, tag="bandp")
            prba_t = cpool.tile([128, 128], f32, tag="prba")
            prbb_t = cpool.tile([128, 512], f32, tag="prbb")
            zrow = cpool.tile([1, 64], f32, tag="zrow")
            nc.sync.dma_start(bandc_t[:], BCD)
            nc.sync.dma_start(bandp_t[:], BPD)
            nc.sync.dma_start(prba_t[:], PAD)
            nc.sync.dma_start(prbb_t[:], PBD)
            nc.gpsimd.memset(zrow[:], 0.0)

            for b in range(BPC):
                tst = spool.tile([128, NT], f32, tag="tst")
                tsh = spool.tile([128, NT], f32, tag="tsh")
                udt = spool.tile([128, NT], f32, tag="udt")
                npt = spool.tile([128, NT], f32, tag="npt")
                nsh = spool.tile([128, NT], f32, tag="nsh")
                nc.sync.dma_start(tst[:], TSD[b, 0:L].rearrange("(n p) -> p n", p=128))
                nc.sync.dma_start(tsh[:], TSD[b, 1:L + 1].rearrange("(n p) -> p n", p=128))
                nc.sync.dma_start(udt[:], UDD[b].rearrange("(n p) -> p n", p=128))
                nc.sync.dma_start(npt[:], NPD[b, 0:L].rearrange("(n p) -> p n", p=128))
                nc.sync.dma_start(nsh[:], NPD[b, 1:L + 1].rearrange("(n p) -> p n", p=128))
                # real row for l=0 is identically zero
                nc.sync.dma_start(OUTD[b, 0:1, :], zrow[:])

                psw_next = None
                for n in range(NT):
                    ftile = fpool.tile([128, C], f32, tag="f")
                    nc.sync.dma_start(ftile[:], FD[b, n * 128:(n + 1) * 128, :])
                    gh = ghpool.tile([128, 128], f32, tag="gh")
                    nc.scalar.activation(gh[:, 0:64], ftile[:], Copy,
                                         scale=npt[:, n:n + 1])
                    nc.vector.tensor_scalar_mul(gh[:, 64:128], gh[:, 0:64],
                                                tst[:, n:n + 1])
                    # MM1: windowed sums, transposed output
                    if n == 0:
                        psw_cur = pwpool.tile([128, 128], f32, tag="psw")
                        nc.tensor.matmul(psw_cur[:], gh[:], bandc_t[:],
                                         start=True, stop=True)
                    else:
                        psw_cur = psw_next
                        nc.tensor.matmul(psw_cur[:], gh[:], bandc_t[:],
                                         start=False, stop=True)
                    if n < NT - 1:
                        psw_next = pwpool.tile([128, 128], f32, tag="psw")
                        nc.tensor.matmul(psw_next[:], gh[:], bandp_t[:],
                                         start=True, stop=False)
                    sbw = sbwpool.tile([128, 128], f32, tag="sbw")
                    nc.scalar.copy(sbw[:], psw_cur[:])
                    # MM2: project windowed features
                    psp = papool.tile([128, 128], f32, tag="psp")
                    nc.tensor.matmul(psp[:], sbw[:], prba_t[:], start=True, stop=True)
                    pssp = pbpool.tile([128, 512], f32, tag="pssp")
                    nc.tensor.matmul(pssp[:], sbw[:], prbb_t[:], start=True, stop=True)
                    pp = pppool.tile([128, 128], f32, tag="pp")
                    nc.scalar.activation(pp[:], psp[:], Copy, scale=npt[:, n:n + 1])
                    sim_m = simpool.tile([128, 64], f32, tag="simm")
                    nc.vector.scalar_tensor_tensor(
                        sim_m[:], pp[:, 0:64], tst[:, n:n + 1], pp[:, 64:128],
                        op0=mult, op1=sub)
                    obsim = obpool.tile([128, 512], f32, tag="ob")
                    nc.vector.scalar_tensor_tensor(
                        obsim[:].rearrange("p (s o) -> p s o", o=64),
                        pssp[:].rearrange("p (s o) -> p s o", o=64),
                        udt[:, n:n + 1],
                        sim_m[:].unsqueeze(1).broadcast_to([128, 8, 64]),
                        op0=mult, op1=add)
                    # real rows for l = l0+1 .. l0+128, lane p -> l0+p+1
                    rr = ropool.tile([128, 64], f32, tag="rr")
                    nc.vector.scalar_tensor_tensor(
                        rr[:], pp[:, 0:64], tsh[:, n:n + 1], pp[:, 64:128],
                        op0=mult, op1=sub)
                    rm = ropool.tile([128, 64], f32, tag="rm")
                    nc.vector.tensor_scalar_mul(rm[:], rr[:], nsh[:, n:n + 1])
                    # store
                    PR = 128 if n < NT - 1 else 127
                    real_dst = bass.AP(
                        OUTD.tensor, (b * ROWS + 9 * (n * 128 + 1)) * 64,
                        [[9 * 64, PR], [1, 64]])
                    nc.sync.dma_start(real_dst, rm[0:PR, :])
                    if n < NT - 1:
                        blk = OUTD[b, 9 * n * 128: 9 * (n + 1) * 128, :] \
                            .rearrange("(p s) o -> p s o", s=9)
                        nc.sync.dma_start(blk[:, 1:9, :],
                                          obsim[:].rearrange("p (s o) -> p s o", o=64))
                    else:
                        blk = OUTD[b, 9 * n * 128: 9 * n * 128 + 9 * 127, :] \
                            .rearrange("(p s) o -> p s o", s=9)
                        nc.sync.dma_start(
                            blk[:, 1:9, :],
                            obsim[0:127, :].rearrange("p (s o) -> p s o", o=64))
    nc.compile()
    return nc


_NC_CACHE = None


def kernel(**inputs):
    global _NC_CACHE
    from concourse.bass_utils import run_bass_kernel_spmd

    times = np.ascontiguousarray(inputs["times"], np.float32)
    feats = np.ascontiguousarray(inputs["features"], np.float32)
    npm = inputs["non_pad_mask"].astype(np.float32)
    u = np.asarray(inputs["uniform_sample"], np.float32)
    W = np.ascontiguousarray(inputs["W"], np.float32)
    bias = np.ascontiguousarray(inputs["bias_param"], np.float32)

    bandc, bandp, prba, prbb = _consts(W, bias, u)
    tnext = np.concatenate([times[:, 1:], np.zeros((B, 1), np.float32)], 1)
    npmn = np.concatenate([npm[:, 1:], np.zeros((B, 1), np.float32)], 1)
    udt = (tnext - times) * npm * npmn  # (B, L); l=L-1 col unused downstream

    if _NC_CACHE is None:
        _NC_CACHE = _build_nc()
    nc = _NC_CACHE

    pad = np.zeros((B, 128), np.float32)
    times_p = np.concatenate([times, pad], 1)
    npm_p = np.concatenate([npm, pad], 1)

    in_maps = []
    for c in range(NCORES):
        sl = slice(c * BPC, (c + 1) * BPC)
        in_maps.append({
            "f": np.ascontiguousarray(feats[sl]),
            "ts": np.ascontiguousarray(times_p[sl]),
            "ud": np.ascontiguousarray(udt[sl]),
            "np": np.ascontiguousarray(npm_p[sl]),
            "bandc": bandc, "bandp": bandp, "prba": prba, "prbb": prbb,
        })
    res = run_bass_kernel_spmd(nc, in_maps, core_ids=list(range(NCORES)))
    out = np.concatenate([r["out"] for r in res.results], 0)
    return out.astype(np.float32)



# revision 12
# speedup vs baseline: 1.4914x; 1.4914x over previous
"""Trainium2 Bass kernel for ContinuousConv1DSim (gnn_message_passing).

Reformulation (validated vs reference, rel err ~3e-5 in f32; bf16 matmuls):
  G = F * npm (per-l mask), H = G * t
  MM1  (PE): psw[c2, l] = sum_j GH[j, c2] * Band[j, l]   -- causal 8-wide window
             sums over l, output TRANSPOSED (channels on partitions), with a
             second accumulating matmul adding the previous tile's halo rows.
  MM2a (PE): psp[l, 0:64]  = A_e   (window(G) @ W^T)
             psp[l, 64:128]= D_raw (window(H) @ W^T - window(G) @ bias)
  MM2b (PE): pssp[l, s*64+o] = u[s] * A_e[l, o]          -- s-expansion on PE
  smraw   = tst*A_e - D_raw          (DVE STT from PSUM)
  sim_m   = npt * smraw              (ACT scale-copy)
  obsim   = pssp * udt + sim_m       (DVE STT, 512 cols)
  rmraw   = tsh*A_e - D_raw          (DVE STT from PSUM)
  rm      = (npt*nsh) * rmraw        (ACT scale-copy)
Output mapping: lane p (l = n*128+p) owns out rows 9l+1 .. 9l+9:
  rows 9l+1..9l+8 = sim slots s=0..7 for l, row 9l+9 = real[l+1].
So each lane stores one contiguous 2304B run; real[0] rows are zeroed once.

Pure data parallel: batch 32 -> 8 cores x 4. All params replicated.
DMA discipline: every dma_start costs ~0.6us serialized on the sync engine,
so: 1 const DMA + 1 scalar DMA + 16 feature loads + 16 fused stores.
"""

import numpy as np

B, L, C, O, S = 32, 2048, 64, 64, 8
NCORES = 8
BPC = B // NCORES          # 4 batches per core
NT = L // 128              # 16 l-tiles per batch
ROWS = (L - 1) * (S + 1) + 1  # 18424
NKIND = 6                  # tst, tsh, udt, npt, nsh, tstn (next-group center)
GRP = 4                    # tiles per centering group


def _consts(W, bias, u):
    n = np.arange(128)
    bandc = ((n[:, None] >= n[None, :] - 7) & (n[:, None] <= n[None, :])).astype(np.float32)
    bandp = (n[:, None] >= n[None, :] + 121).astype(np.float32)
    prba = np.zeros((128, 128), np.float32)
    prba[0:64, 0:64] = W.T           # A_e from G-window
    prba[0:64, 64:128] = -bias       # -F_e into D_raw
    prba[64:128, 64:128] = W.T       # TA_e into D_raw
    prbb = np.zeros((128, 512), np.float32)
    for s in range(S):
        prbb[0:64, s * 64:(s + 1) * 64] = u[s] * W.T
    # pack [bandc | bandp | prba | prbb] -> [128, 896] f32 (matmuls run f32r)
    return np.concatenate([bandc, bandp, prba, prbb], axis=1)


def make_in_maps(inputs):
    times = np.ascontiguousarray(inputs["times"], np.float32)
    feats = np.ascontiguousarray(inputs["features"], np.float32)
    npm = inputs["non_pad_mask"].astype(np.float32)
    u = np.asarray(inputs["uniform_sample"], np.float32)
    W = np.ascontiguousarray(inputs["W"], np.float32)
    bias = np.ascontiguousarray(inputs["bias_param"], np.float32)

    cpk = _consts(W, bias, u)
    z1 = np.zeros((B, 1), np.float32)
    tnext = np.concatenate([times[:, 1:], z1], 1)
    npmn = np.concatenate([npm[:, 1:], z1], 1)
    udt = (tnext - times) * npm * npmn      # (B, L)

    # Per-(4-tile group) time centering: delta = t_l - t_j is shift
    # invariant, and centered t' keeps the f32r-rounded H = G*t' small
    # enough that the t'*A - D cancellation stays accurate.
    # c[b, g] = times[b, g*512 + 256]
    ngrp = NT // GRP
    c = times[:, (np.arange(ngrp) * GRP * 128 + GRP * 64)]          # (B, ngrp)
    cl = np.repeat(c, GRP * 128, axis=1)                            # center for l's tile
    cn = np.repeat(np.concatenate([c[:, 1:], c[:, -1:]], 1), GRP * 128, axis=1)
    tst = times - cl
    tsh = tnext - cl
    tstn = times - cn
    # scalars: scl[b][p, kind*16 + n] = arr[b, n*128 + p] ; kinds packed per b
    # host layout SCL[p, (b*NKIND + kind)*NT + n]
    kinds = np.stack([tst, tsh, udt, npm, npmn, tstn], axis=1)     # (B, K, L)
    kinds = kinds.reshape(B, NKIND, NT, 128).transpose(0, 3, 1, 2)  # (B, p, K, n)
    # features: fpk[b][n, p, c]
    fpk = feats.reshape(B, NT, 128, C)

    in_maps = []
    for c in range(NCORES):
        sl = slice(c * BPC, (c + 1) * BPC)
        scl = kinds[sl].transpose(1, 0, 2, 3).reshape(128, BPC * NKIND * NT)
        f4 = fpk[sl].transpose(1, 2, 0, 3).reshape(NT, 128, BPC * C)
        in_maps.append({
            "f": np.ascontiguousarray(f4),
            "scl": np.ascontiguousarray(scl),
            "cpk": cpk,
        })
    return in_maps


def _build_nc():
    import concourse.bass as bass
    import concourse.bacc as bacc
    import concourse.mybir as mybir
    import concourse.tile as tile

    f32 = mybir.dt.float32
    f32r = mybir.dt.float32r
    Copy = mybir.ActivationFunctionType.Copy
    mult = mybir.AluOpType.mult
    sub = mybir.AluOpType.subtract
    add = mybir.AluOpType.add

    nc = bacc.Bacc("TRN2", target_bir_lowering=False, debug=False,
                   num_devices=NCORES)

    FD = nc.dram_tensor("f", [NT, 128, BPC * C], f32, kind="ExternalInput").ap()
    SCD = nc.dram_tensor("scl", [128, BPC * NKIND * NT], f32,
                         kind="ExternalInput").ap()
    CPD = nc.dram_tensor("cpk", [128, 896], mybir.dt.float32r, kind="ExternalInput").ap()
    OUTD = nc.dram_tensor("out", [BPC * ROWS * O], f32,
                          kind="ExternalOutput").ap()

    def scol(b, kind, n):
        return (b * NKIND + kind) * NT + n

    with tile.TileContext(nc) as tc:
        with (
            tc.tile_pool(name="const", bufs=1) as cpool,
            tc.tile_pool(name="feat", bufs=3) as fpool,
            tc.tile_pool(name="gh", bufs=2) as ghpool,
            tc.tile_pool(name="sbw", bufs=2) as sbwpool,
            tc.tile_pool(name="sm", bufs=2) as smpool,
            tc.tile_pool(name="ob", bufs=3) as obpool,
            tc.tile_pool(name="psw", bufs=1, space=bass.MemorySpace.PSUM) as pwpool,
            tc.tile_pool(name="psp", bufs=2, space=bass.MemorySpace.PSUM) as papool,
            tc.tile_pool(name="pssp", bufs=2, space=bass.MemorySpace.PSUM) as pbpool,
        ):
            cpk = cpool.tile([128, 896], f32r, tag="cpk")
            scl = cpool.tile([128, BPC * NKIND * NT], f32, tag="scl")
            zrow = cpool.tile([BPC, O], f32, tag="zrow")
            nc.sync.dma_start(cpk[:], CPD)
            nc.sync.dma_start(scl[:], SCD)
            nc.gpsimd.memset(zrow[:], 0.0)
            # real[0] = 0 for each batch (out row b*ROWS + 0)
            zdst = bass.AP(OUTD.tensor, 0, [[ROWS * O, BPC], [1, O]])
            nc.sync.dma_start(zdst, zrow[:])

            bandc = cpk[:, 0:128]
            bandp = cpk[:, 128:256]
            prba = cpk[:, 256:384]
            prbb = cpk[:, 384:896]

            # PSUM is 8 banks x 2KB: psw one bank per batch (start=True
            # clears has_written for the WHOLE bank, so the cross-tile halo
            # accumulation cannot share banks between batches), psp and pssp
            # rotate over 2 banks each: 4 + 2 + 2 = 8.
            psw = [pwpool.tile([128, 128], f32, tag=f"psw{b}", name=f"psw{b}")
                   for b in range(BPC)]
            for n in range(NT):
                f4 = fpool.tile([128, BPC * C], f32, tag="f4")
                nc.sync.dma_start(f4[:], FD[n])
                ob = obpool.tile([128, BPC * 576], f32, tag="ob")
                boundary = (n % GRP == GRP - 1) and n < NT - 1
                for b in range(BPC):
                    # gh cols: 0:64 G, 64:128 H_cur (center c_g), 128:192
                    # H_next (center c_{g+1}, only built at group boundaries)
                    gh = ghpool.tile([128, 192], f32r, tag=f"gh{b}")
                    nc.scalar.activation(gh[:, 0:64], f4[:, b * C:(b + 1) * C],
                                         Copy, scale=scl[:, scol(b, 3, n):scol(b, 3, n) + 1])
                    nc.vector.tensor_scalar_mul(gh[:, 64:128], gh[:, 0:64],
                                                scl[:, scol(b, 0, n):scol(b, 0, n) + 1])
                    if boundary:
                        nc.vector.tensor_scalar_mul(gh[:, 128:192], gh[:, 0:64],
                                                    scl[:, scol(b, 5, n):scol(b, 5, n) + 1])
                    # MM1: windowed sums, transposed output.  The halo from
                    # tile n-1 was started into cols 0:7 by the narrow bandp
                    # matmul below (start=True); bandc accumulates there and
                    # overwrites cols 7:128 (has_written clear).
                    psw_cur = psw[b][:]
                    nc.tensor.matmul(psw_cur, gh[:, 0:128], bandc,
                                     start=(n == 0), stop=True,
                                     skip_group_check=True)
                    sbw = sbwpool.tile([128, 128], f32r, tag=f"sbw{b}")
                    nc.scalar.copy(sbw[:], psw_cur)
                    if n < NT - 1:
                        if boundary:
                            # halo H must use the NEXT group's center: split
                            # into G-rows and H_next-rows (plain f32, tiny)
                            ghf = gh[:].bitcast(f32)
                            bpf = bandp[:, 0:8].bitcast(f32)
                            nc.tensor.matmul(psw[b][0:64, 0:8], ghf[:, 0:64],
                                             bpf, start=True, stop=False,
                                             skip_group_check=True)
                            nc.tensor.matmul(psw[b][64:128, 0:8],
                                             ghf[:, 128:192],
                                             bpf, start=True, stop=False,
                                             skip_group_check=True)
                        else:
                            nc.tensor.matmul(psw[b][:, 0:8], gh[:, 0:128],
                                             bandp[:, 0:8],
                                             start=True, stop=False,
                                             skip_group_check=True)
                    # MM2: project windowed features
                    psp = papool.tile([128, 128], f32, tag="psp")
                    nc.tensor.matmul(psp[:], sbw[:], prba, start=True, stop=True)
                    pssp = pbpool.tile([128, 512], f32, tag="pssp")
                    nc.tensor.matmul(pssp[:], sbw[:], prbb, start=True, stop=True)
                    # pp = npt * psp = A_m | D_m  (PSUM -> SBUF, DVE reads SBUF)
                    pp = smpool.tile([128, 128], f32, tag=f"pp{b}")
                    nc.scalar.activation(pp[:], psp[:], Copy,
                                         scale=scl[:, scol(b, 3, n):scol(b, 3, n) + 1])
                    # sim rows
                    sim_m = smpool.tile([128, 64], f32, tag=f"simm{b}")
                    nc.vector.scalar_tensor_tensor(
                        sim_m[:], pp[:, 0:64],
                        scl[:, scol(b, 0, n):scol(b, 0, n) + 1],
                        pp[:, 64:128], op0=mult, op1=sub)
                    nc.vector.scalar_tensor_tensor(
                        ob[:, b * 576:b * 576 + 512].rearrange(
                            "p (s o) -> p s o", o=O),
                        pssp[:].rearrange("p (s o) -> p s o", o=O),
                        scl[:, scol(b, 2, n):scol(b, 2, n) + 1],
                        sim_m[:].unsqueeze(1).broadcast_to([128, S, O]),
                        op0=mult, op1=add)
                    # real row for l+1: rr = tsh*A_m - D_m ; rm = nsh*rr
                    rr = smpool.tile([128, 64], f32, tag=f"rr{b}")
                    nc.vector.scalar_tensor_tensor(
                        rr[:], pp[:, 0:64],
                        scl[:, scol(b, 1, n):scol(b, 1, n) + 1],
                        pp[:, 64:128], op0=mult, op1=sub)
                    nc.scalar.activation(ob[:, b * 576 + 512:(b + 1) * 576],
                                         rr[:], Copy,
                                         scale=scl[:, scol(b, 4, n):scol(b, 4, n) + 1])
                # one fused store: lane p -> rows 9*(n*128+p)+1 .. +9, all batches
                PR = 128 if n < NT - 1 else 127
                dst = bass.AP(OUTD.tensor, (9 * n * 128 + 1) * O,
                              [[9 * O, PR], [ROWS * O, BPC], [1, 576]])
                nc.sync.dma_start(
                    dst, ob[0:PR, :].rearrange("p (b x) -> p b x", b=BPC))
    nc.compile()
    return nc


_NC_CACHE = None


def kernel(**inputs):
    global _NC_CACHE
    from concourse.bass_utils import run_bass_kernel_spmd

    if _NC_CACHE is None:
        _NC_CACHE = _build_nc()
    nc = _NC_CACHE

    in_maps = make_in_maps(inputs)
    res = run_bass_kernel_spmd(nc, in_maps, core_ids=list(range(NCORES)))
    out = np.concatenate(
        [r["out"].reshape(BPC, ROWS, O) for r in res.results], 0)
    return out.astype(np.float32)
